# revision 1
# baseline (speedup 1.0000x reference)
"""Multi-head attention (B=2, H=16, S=2048, D=64) on 8 trn2 NeuronCores.

Sharding: the 32 (b, h) head-units are split 4-per-core (head/data parallel,
no cross-core comms).  Per core, for each head:

  scoresT[k, q] = sum_d K[k, d] Q[q, d]            (PE, contract=64, row-packed 2x)
  pT[k, q]      = masked-exp(scoresT / 8)          (split across two engines, see
                                                    below)
  OT'[m, q]     = sum_k V'[k, m] pT[k, q]          (PE, V' = [V | ones] so row 64
                                                    of OT' is the softmax denom Z)
  out[q, d]     = OT'[d, q] / OT'[64, q]           (host-side: O(S*D) divide +
                                                    transpose while unsharding)

The exp+mask stage is the throughput bottleneck (16.8M elements/core with
every engine limited to 128 lanes/cycle), so it is split across the two
elementwise engines to run in parallel:

  * ACT chunks (k-chunks 0..11): nc.scalar.activation Exp at 1 elem/cyc
    @1.2 GHz, then a DVE fp16 tensor_tensor multiply with the 0/1 keep mask
    (2x_1P mode, 0.5 cyc/elem @0.96 GHz).
  * DVE chunks (k-chunks 12..15): a single fused scalar_tensor_tensor op
    computes uint16 t = round(raw*184.665 + B) saturating at [0, 65535];
    the uint16 bit pattern *is* fp16 exp(raw/8) (Schraudolph bit trick,
    ~1% softmax error) and B encodes the mask additively (keep: 15328 =
    Schraudolph bias, masked: -60000 -> t saturates to 0 -> fp16 +0.0).
    One 1-elem/cyc DVE pass replaces the exp AND the mask multiply.

Working in the transposed-score layout means softmax needs no reductions at
all (Z rides along in the PV matmul) and no S x S transposes anywhere.

Host-side (numpy, not on the critical HW path): Q/K are passed pre-transposed
per head as [64, S]; V is passed chunk-interleaved fp16 with the ones column
appended; the shared mask is passed transposed, chunk-major, with ACT chunks
encoded as 0/1 fp16 and DVE chunks encoded as the Schraudolph bias tensor.
"""

import numpy as np

import concourse.bass as bass  # noqa: F401  (engine types resolve through nc)
import concourse.mybir as mybir
import concourse.tile as tile
from concourse import bacc
from concourse.bass_utils import run_bass_kernel_spmd

B, H, S, D = 2, 16, 2048, 64
N_CORES = 8
HPC = (B * H) // N_CORES  # heads per core

SQ = 512        # query-block width (one fp32 PSUM bank)
CK = 128        # key-chunk height (PSUM partition dim)
# (start chunk, n chunks, engine) per exp group, in issue order.  2-chunk
# groups x 3 rotating PSUM slots (6 banks) mean QK group g's slot reuse
# waits on the exp of group g-3 — two QK groups plus an interleaved 4-chunk
# PV block (~1.6us of PE work) earlier, so the in-order PE queue never
# idles on ACT/DVE.  The DVE (Schraudolph) groups sit mid-unit: their slot
# heirs have PV cover while the DVE queue (which drains trailing masks of
# the previous unit first) catches up.
GROUPS = [(0, 2, "act"), (2, 2, "act"), (4, 2, "act"), (12, 2, "dve"),
          (14, 2, "dve"), (6, 2, "act"), (8, 2, "act"), (10, 2, "act")]
QK_BUFS = 3     # qk PSUM slots (2 banks each) + 2 o_ps accumulators = 8 banks
NACT = 12       # chunks 0..NACT-1 -> ACT exp path, rest -> DVE Schraudolph
VW = D + 2      # V' width: 64 V columns + ones column + pad (66)

A_SCH = 184.664964          # 1024 * log2(e) / 8
B_KEEP = 15328.0            # 15*1024 + 1024*corr, corr=-0.03125 (minimax-ish)
B_MASK = -60000.0           # saturates uint16 convert to 0 -> fp16 +0.0

f32 = mybir.dt.float32
f16 = mybir.dt.float16
u16 = mybir.dt.uint16
FT = mybir.ActivationFunctionType


def build_nc(hpc=HPC, s=S, loop_n=None, ablate=(), loop_stagger=False):
    """Build the per-core Bass program (identical on all 8 cores).

    loop_n: if set, wrap the whole body in an on-device For_i loop that
    recomputes the same output loop_n times — a perf-measurement rig that
    lets wall-clock deltas between two loop_n values cancel host/RPC
    overheads (this container has no NTFF profile path).

    ablate: perf-debug only — subset of {"qk", "act", "mask", "pv", "tail"}
    to skip emitting, isolating per-engine throughput on HW. Output is
    garbage when non-empty.
    """
    nsq = s // SQ
    nck = s // CK
    if nck == 16:
        groups = list(GROUPS)
    else:  # small-s debug builds: every chunk on ACT
        groups = [(c, 1, "act") for c in range(nck)]
    ablate = set(ablate)

    nc = bacc.Bacc("TRN2", target_bir_lowering=False, debug=False)

    qt_d = nc.dram_tensor("qt", [hpc, D, s], f16, kind="ExternalInput")
    kt_d = nc.dram_tensor("kt", [hpc, D, s], f16, kind="ExternalInput")
    vp_d = nc.dram_tensor("vp", [hpc, CK, nck * VW], f16, kind="ExternalInput")
    mk_d = nc.dram_tensor("mk", [nsq, CK, nck * SQ], f16, kind="ExternalInput")
    o_d = nc.dram_tensor("o", [hpc, nsq, VW, SQ], f16, kind="ExternalOutput")

    with tile.TileContext(nc) as tc:
        if ablate:
            tc.race_detector_enabled = False
        with (
            tc.tile_pool(name="heads", bufs=hpc) as head_pool,
            tc.tile_pool(name="mask", bufs=nsq) as mask_pool,
            # bufs=3: with 2, unit k's first exp would overwrite the pt slot
            # that unit k-2's interleaved PV matmuls are still reading,
            # serializing ACT behind PE.
            tc.tile_pool(name="pt", bufs=3) as pt_pool,
            tc.tile_pool(name="tail", bufs=2) as tail_pool,
            tc.tile_pool(name="qk_ps", bufs=QK_BUFS, space="PSUM") as qk_pool,
            tc.tile_pool(name="o_ps", bufs=2, space="PSUM") as o_pool,
        ):
            qt_t, kt_t, vp_t = [], [], []
            for h in range(hpc):
                q_t = head_pool.tile([128, s], f16, name=f"qt_sb{h}", tag="qt")
                k_t = head_pool.tile([128, s], f16, name=f"kt_sb{h}", tag="kt")
                v_t = head_pool.tile([CK, nck * VW], f16, name=f"vp_sb{h}", tag="vp")
                # Q^T/K^T live duplicated in both partition halves so the two
                # row-packed K=64 matmuls can run concurrently on the PE.
                nc.sync.dma_start(out=q_t[0:D, :], in_=qt_d[h, :, :])
                nc.sync.dma_start(out=q_t[D:128, :], in_=qt_d[h, :, :])
                nc.sync.dma_start(out=k_t[0:D, :], in_=kt_d[h, :, :])
                nc.sync.dma_start(out=k_t[D:128, :], in_=kt_d[h, :, :])
                nc.sync.dma_start(out=v_t[:, :], in_=vp_d[h, :, :])
                qt_t.append(q_t)
                kt_t.append(k_t)
                vp_t.append(v_t)

            # The whole mask fits in SBUF — load it once, outside any
            # measurement loop (saves 8MB of DMA per pass).
            mk_t = {}     # sqb -> mask tile [128, nck*SQ] (chunk-major columns)
            for sqb in range(nsq):
                mk = mask_pool.tile([CK, nck * SQ], f16, name=f"mk_sb{sqb}",
                                    tag="mk")
                nc.sync.dma_start(out=mk[:, :], in_=mk_d[sqb, :, :])
                mk_t[sqb] = mk

            pt_t = {}     # (sqb, h) -> p^T tile [128, nck*SQ] fp16
            o_ps = {}     # (sqb, h) -> PSUM accumulator [VW, SQ]

            def emit_qk_group(sqb, h, c0, n, kind):
                """QK matmuls + exp (ACT) or fused Schraudolph (DVE) for
                chunks [c0, c0+n)."""
                qk = None
                if "qk" not in ablate:
                    qk = qk_pool.tile([128, n * SQ], f32,
                                      name=f"qk_{sqb}_{h}_{c0}", tag="qk",
                                      padded_shape=[128, 2 * SQ])
                for j in range(n):
                    if "qk" in ablate:
                        break
                    c = c0 + j
                    bp = 64 * (c % 2)  # row-group for PE packing (global
                    # chunk parity so consecutive issues alternate rows and
                    # run concurrently even across group boundaries)
                    nc.tensor.matmul(
                        qk[:, j * SQ:(j + 1) * SQ],
                        lhsT=kt_t[h][bp:bp + D, c * CK:(c + 1) * CK],
                        rhs=qt_t[h][bp:bp + D, sqb * SQ:(sqb + 1) * SQ],
                        start=True,
                        stop=True,
                        tile_position=(bp, 0),
                    )
                pt = pt_t[(sqb, h)]
                lo = c0 * SQ
                hi = (c0 + n) * SQ
                if "act" in ablate or pt is None:
                    return
                act_in = qk[:, :] if qk is not None else mk_t[sqb][:, lo:hi]
                if kind == "act":
                    nc.scalar.activation(pt[:, lo:hi], act_in, FT.Exp,
                                         scale=0.125)
                else:
                    nc.vector.scalar_tensor_tensor(
                        pt.bitcast(u16)[:, lo:hi], act_in, A_SCH,
                        mk_t[sqb][:, lo:hi],
                        op0=mybir.AluOpType.mult, op1=mybir.AluOpType.add,
                    )

            def emit_mask(sqb, h, clo, chi):
                """Apply the 0/1 keep-mask to ACT-path chunk cols [clo, chi)
                of p^T in one fp16 2x-mode DVE pass."""
                if "mask" in ablate:
                    return
                clo, chi = min(clo, NACT), min(chi, NACT)
                if clo >= chi:
                    return
                pt = pt_t[(sqb, h)]
                lo, hi = clo * SQ, chi * SQ
                nc.vector.tensor_tensor(
                    pt[:, lo:hi], pt[:, lo:hi], mk_t[sqb][:, lo:hi],
                    op=mybir.AluOpType.mult,
                )

            def emit_pv(sqb, h, clo, chi):
                """PV matmuls for chunks [clo, chi), accumulating."""
                if "pv" in ablate:
                    return
                pt = pt_t[(sqb, h)]
                if pt is None:
                    pt = mk_t[sqb]  # stand-in written tile for PE-only ablations
                ops = o_ps[(sqb, h)]
                for c in range(clo, chi):
                    nc.tensor.matmul(
                        ops[:, :],
                        lhsT=vp_t[h][:, c * VW:c * VW + VW],
                        rhs=pt[:, c * SQ:(c + 1) * SQ],
                        start=(c == 0),
                        stop=(c == nck - 1),
                    )

            def emit_tail(sqb, h):
                """Evacuate O^T' (unnormalized + Z row) as fp16 and store.
                On ACT: DVE is the busier elementwise engine."""
                if "tail" in ablate or "pv" in ablate:
                    return
                ops = o_ps[(sqb, h)]
                ot = tail_pool.tile([VW, SQ], f16, name=f"ot_{sqb}_{h}", tag="ot")
                nc.scalar.copy(ot[:, :], ops[:, :])
                nc.sync.dma_start(out=o_d[h, sqb, :, :], in_=ot[:, :])

            def emit_alloc(sqb, h):
                if not ({"act", "mask"} <= ablate):
                    pt_t[(sqb, h)] = pt_pool.tile(
                        [128, nck * SQ], f16, name=f"pt_{sqb}_{h}", tag="pt")
                else:
                    pt_t[(sqb, h)] = None
                if "pv" not in ablate:
                    o_ps[(sqb, h)] = o_pool.tile(
                        [VW, SQ], f32, name=f"ops_{sqb}_{h}", tag="ops")

            def emit_unit(k, units):
                """Fine-grained interleave: unit k's QK groups with unit
                k-2's PV blocks, so the in-order PE queue always has PV work
                in hand while an exp group it depends on (via PSUM-slot
                reuse) is still running on ACT/DVE."""
                u = units[k]
                prev = units[k - 2] if k >= 2 else None
                emit_alloc(*u)
                if nck != 16:   # debug path: simple sequential
                    for c0, n, kind in groups:
                        emit_qk_group(*u, c0, n, kind)
                    emit_mask(*u, 0, nck)
                    if prev:
                        emit_drain(k - 2, units)
                    return
                # groups: a0(c0-1) a1(c2-3) a2(c4-5) d0(c12-13) d1(c14-15)
                #         a3(c6-7) a4(c8-9) a5(c10-11)
                # Two PV blocks only: each QK<->PV switch on the PE exposes a
                # full-array LDWEIGHTS (row groups conflict, no pull-ahead),
                # so fewer, bigger blocks beat fine interleave.
                emit_qk_group(*u, *groups[0])
                emit_qk_group(*u, *groups[1])
                emit_qk_group(*u, *groups[2])
                if prev:
                    emit_pv(*prev, 0, 8)
                emit_qk_group(*u, *groups[3])
                emit_qk_group(*u, *groups[4])
                emit_qk_group(*u, *groups[5])
                if prev:
                    emit_pv(*prev, 8, nck)
                    emit_tail(*prev)
                emit_qk_group(*u, *groups[6])
                emit_qk_group(*u, *groups[7])
                # Masks trail: they keep DVE busy through the next unit's QK
                # phase and nothing reads pt chunks 0-11 for 2 more units.
                emit_mask(*u, 0, 4)
                emit_mask(*u, 4, 8)
                emit_mask(*u, 8, 12)

            def emit_drain(k, units):
                u = units[k]
                emit_pv(*u, 0, nck)
                emit_tail(*u)

            def emit_all():
                units = [(sqb, h) for sqb in range(nsq) for h in range(hpc)]
                for k in range(len(units)):
                    emit_unit(k, units)
                for k in (len(units) - 2, len(units) - 1):
                    if k >= 0:
                        emit_drain(k, units)

            if loop_n is None:
                emit_all()
            else:
                hints = (mybir.EngineType.PE, mybir.EngineType.Activation,
                         mybir.EngineType.DVE)
                with tc.For_i(0, loop_n, 1, hint_engines=hints,
                              staggered_reset=bool(loop_stagger)):
                    emit_all()

    nc.finalize()
    return nc


def shard_inputs(K, Q, V, mask, hpc=HPC, s=S, n_cores=N_CORES):
    """Full inputs -> per-core in_maps with device-friendly host layouts."""
    nsq = s // SQ
    nck = s // CK
    n_units = n_cores * hpc
    Kf = np.asarray(K, np.float32).reshape(n_units, s, D)
    Qf = np.asarray(Q, np.float32).reshape(n_units, s, D)
    Vf = np.asarray(V, np.float32).reshape(n_units, s, D)
    keepT = (~np.asarray(mask).reshape(s, s)).T  # [k, q], True = attend
    # ACT chunks: 0/1 multiplier.  DVE chunks: Schraudolph additive bias.
    nact = NACT if nck == 16 else nck
    mk_f = keepT.astype(np.float16)              # [k, q] 0/1
    mk_f = mk_f.reshape(nck, CK, s)
    dve = np.where(keepT.reshape(nck, CK, s)[nact:] > 0, np.float16(B_KEEP),
                   np.float16(B_MASK))
    mk_full = np.concatenate([mk_f[:nact], dve], axis=0)  # [nck, CK, s]
    mk_host = np.ascontiguousarray(
        mk_full.reshape(nck, CK, nsq, SQ)
        .transpose(2, 1, 0, 3)
        .reshape(nsq, CK, nck * SQ)
    )
    in_maps = []
    for c in range(n_cores):
        sl = slice(c * hpc, (c + 1) * hpc)
        qt = np.ascontiguousarray(Qf[sl].transpose(0, 2, 1)).astype(np.float16)
        kt = np.ascontiguousarray(Kf[sl].transpose(0, 2, 1)).astype(np.float16)
        vp = np.zeros((hpc, s, VW), np.float16)
        vp[:, :, :D] = Vf[sl]
        vp[:, :, D] = 1.0
        vp = np.ascontiguousarray(
            vp.reshape(hpc, nck, CK, VW).transpose(0, 2, 1, 3)
            .reshape(hpc, CK, nck * VW)
        )
        in_maps.append({"qt": qt, "kt": kt, "vp": vp, "mk": mk_host})
    return in_maps


_NC_CACHE = {}


def _get_nc():
    if "nc" not in _NC_CACHE:
        _NC_CACHE["nc"] = build_nc()
    return _NC_CACHE["nc"]


def run_sharded(in_maps, trace=False, **kwargs):
    return run_bass_kernel_spmd(
        _get_nc(), in_maps, core_ids=list(range(N_CORES)), trace=trace, **kwargs
    )


def unshard_output(per_core_raw, hpc=HPC, s=S):
    """[hpc, nsq, VW, SQ] raw blocks per core -> [n*hpc, s, D] normalized.

    Row D of each block is the softmax denominator Z; dividing and
    transposing here is O(S*D) host work (same order as unsharding).
    """
    n = len(per_core_raw)
    out = np.empty((n * hpc, s, D), np.float32)
    for c, o in enumerate(per_core_raw):
        of = np.asarray(o, np.float32)              # raw blocks arrive fp16
        ot = of[:, :, :D, :] / of[:, :, D:D + 1, :]  # [hpc, nsq, D, SQ]
        out[c * hpc:(c + 1) * hpc] = (
            ot.transpose(0, 1, 3, 2).reshape(hpc, s, D))
    return out


def assemble_output(results):
    out = unshard_output([results[c]["o"] for c in range(N_CORES)])
    return out.reshape(B, H, S, D)


def kernel(K, Q, V, mask):
    in_maps = shard_inputs(K, Q, V, mask)
    res = run_sharded(in_maps)
    return assemble_output(res.results)

